# revision 41
# baseline (speedup 1.0000x reference)
"""Trainium2 Bass kernel for nn_AutoDecoder (moe_routing).

Reference computation (per full input):
  x: [S=3072, B=32, C=512]; rows s%3==1 are "brick" tokens, s%3==2 are
  "combined" tokens (s%3==0 PAD rows are dead). For each (timestep, batch)
  pair:
    brick:  logits[0:80]    = x_brick @ [Ws|Wc]            (+ biases)
    comb:   h = relu(relu(x_comb @ W1 + b1) @ W2 + b2)
            logits[80:1000] = h @ Wh + bh
  out: [TS=1024, B=32, A=1000]

Strategy: data-parallel over batch (4 batch entries per core, 8 cores),
weights replicated. All heavy lifting that is NOT matmul is moved to the
host: x is pre-transposed to feature-major fp16 per readout name, so the
device never transposes, and the MLP + heads are pure back-to-back fp16
matmuls with fp32 PSUM accumulation.

Per core, tokens are processed in blocks of up to 512 (ts,b) pairs:
  L1/L2: feature-major h = relu(W.T @ xT) via ACT (bias fused).
  comb head: token-major logits, stationary = h2 column slices, moving =
    Wh columns (920 streamed cols, no padding waste); DVE adds the
    free-dim bias and narrows to fp16.
  brick head: feature-major [80, W] with Wsc stationary -- 4 matmuls per
    block instead of 16 LDW-bound per-tile matmuls; it also fills the PE
    pipe while the last h2 activation drains. ACT adds the per-partition
    bias. The host transposes the [80, ntok] result back.
Outputs are written fp16 (halves write traffic); the host upcasts.
"""
import sys

if "/opt/trn_rl_repo" not in sys.path:
    sys.path.append("/opt/trn_rl_repo")

import numpy as np

import concourse.bass as bass
from concourse import bacc
import concourse.mybir as mybir
import concourse.tile as tile
from concourse.bass import ts
from concourse.bass_utils import run_bass_kernel_spmd

F32 = mybir.dt.float32
F16 = mybir.dt.float16
RELU = mybir.ActivationFunctionType.Relu
IDENT = mybir.ActivationFunctionType.Identity

# problem dims (hardcoded; kernel.py must be self-contained)
S, B, C = 3072, 32, 512
TS_ = S // 3                      # 1024 timesteps
NUM_SHAPES, NUM_COLORS, N_COMBINED = 64, 16, 920
NBRICK = NUM_SHAPES + NUM_COLORS  # 80
A = NBRICK + N_COMBINED           # 1000
NCORES = 8
BL = B // NCORES                  # 4 batch entries per core
NTOK = TS_ * BL                   # 4096 tokens per name per core
TT = 128                          # tokens per tok-tile
TPB = TT // BL                    # 32 timesteps per tok-tile
KC = C // 128                     # 4 contraction chunks

_BUILD_CACHE = {}


def _build():
    if "nc" in _BUILD_CACHE:
        return _BUILD_CACHE["nc"]
    nc = bacc.Bacc("TRN2", target_bir_lowering=False, debug=False)

    xc_d = nc.declare_dram_parameter("xc", [128, KC, NTOK], F16, isOutput=False)
    xb_d = nc.declare_dram_parameter("xb", [128, KC, NTOK], F16, isOutput=False)
    w1_d = nc.declare_dram_parameter("w1", [128, KC, C], F16, isOutput=False)
    w2_d = nc.declare_dram_parameter("w2", [128, KC, C], F16, isOutput=False)
    wh_d = nc.declare_dram_parameter("wh", [128, KC, N_COMBINED], F16, isOutput=False)
    wsc_d = nc.declare_dram_parameter("wsc", [128, KC, NBRICK], F16, isOutput=False)
    b1_d = nc.declare_dram_parameter("b1t", [128, KC], F32, isOutput=False)
    b2_d = nc.declare_dram_parameter("b2t", [128, KC], F32, isOutput=False)
    bco_d = nc.declare_dram_parameter("bco", [128, N_COMBINED], F16, isOutput=False)
    bbr_d = nc.declare_dram_parameter("bbr", [NBRICK, 1], F32, isOutput=False)
    oc_d = nc.declare_dram_parameter("oc", [TS_, BL, N_COMBINED], F16, isOutput=True)
    ob_d = nc.declare_dram_parameter("ob", [NBRICK, NTOK], F16, isOutput=True)

    with tile.TileContext(nc) as tc:
        with (
            tc.tile_pool(name="const", bufs=1) as const,
            tc.tile_pool(name="xin", bufs=2) as xin_p,
            tc.tile_pool(name="h", bufs=2) as h_p,
            tc.tile_pool(name="osb", bufs=3) as o_p,
            tc.tile_pool(name="occ", bufs=4) as oc_p,
            tc.tile_pool(name="psh", bufs=3, space=bass.MemorySpace.PSUM) as ps_h,
            tc.tile_pool(name="psb", bufs=1, space=bass.MemorySpace.PSUM) as ps_b,
            tc.tile_pool(name="psc", bufs=2, space=bass.MemorySpace.PSUM) as ps_c,
        ):
            # ---- main loop: blocks of nt tok-tiles (128 tokens each) ----
            # Ramp with W=384 blocks: less DMA demand up front, and N>=384
            # matmuls still hide their LDWEIGHTS (unlike N=256).
            sched = [3, 3, 3, 3, 4, 4, 4, 4, 4]
            assert sum(sched) == TS_ // TPB
            NBLK = len(sched)
            block_in = {}

            def issue_inputs(bi, names=(0, 1)):
                # one DMA per name: [128 c-part, KC chunks, W tokens] fp16.
                # xc on the sync queue, xb on gpsimd: parallel issue.
                nt = sched[bi]
                W_ = nt * TT
                k0 = sum(sched[:bi]) * TT
                pair = block_in.setdefault(bi, [None, None])
                if 0 in names:
                    xc_t = xin_p.tile([128, KC, W_], F16, tag="xc")
                    nc.sync.dma_start(xc_t[:], xc_d[:, :, k0 : k0 + W_])
                    pair[0] = xc_t
                if 1 in names:
                    xb_t = xin_p.tile([128, KC, W_], F16, tag="xb")
                    nc.gpsimd.dma_start(xb_t[:], xb_d[:, :, k0 : k0 + W_])
                    pair[1] = xb_t
            # DMA issue order tracks first-use time under the skewed ramp:
            # w1, xc0, xc1, w2, xb0, wsc, wh, xb1, biases. Keeps every
            # transfer ahead of its consumer while the rings are saturated.
            w1_sb = const.tile([128, KC, C], F16, tag="w1")
            nc.scalar.dma_start(w1_sb[:], w1_d[:])
            issue_inputs(0, names=(0,))
            issue_inputs(1, names=(0,))
            w2_sb = const.tile([128, KC, C], F16, tag="w2")
            nc.scalar.dma_start(w2_sb[:], w2_d[:])
            issue_inputs(0, names=(1,))
            wsc_sb = const.tile([128, KC, NBRICK], F16, tag="wsc")
            nc.scalar.dma_start(wsc_sb[:], wsc_d[:])
            wh_sb = const.tile([128, KC, N_COMBINED], F16, tag="wh")
            nc.scalar.dma_start(wh_sb[:], wh_d[:])
            issue_inputs(1, names=(1,))
            b12_sb = const.tile([128, 2 * KC], F32, tag="b12")
            nc.scalar.dma_start(b12_sb[:, 0:KC], b1_d[:, :])
            nc.scalar.dma_start(b12_sb[:, KC : 2 * KC], b2_d[:, :])
            b1_sb = b12_sb[:, 0:KC]
            b2_sb = b12_sb[:, KC : 2 * KC]
            bco_sb = const.tile([128, N_COMBINED], F16, tag="bco")
            nc.scalar.dma_start(bco_sb[:], bco_d[:, :])
            bbr_sb = const.tile([NBRICK, 1], F32, tag="bbr")
            nc.scalar.dma_start(bbr_sb[:], bbr_d[:, :])

            # ---- HAM warmup: keep the PE continuously busy until w1 + xc0
            # have landed (~+15us) so the clock gate is released (K=8/8) and
            # the first real matmuls run warm with no leading gap (a ramp
            # gap re-throttles the PE to half clock). ----
            warm_src = const.tile([128, 512], F16, tag="warm")
            nc.vector.memset(warm_src[:], 0.0)
            # pre-fire the one-time ACT activation-table load
            warm_act = const.tile([1, 1], F32, tag="warmact")
            nc.scalar.activation(warm_act[:], warm_src[0:1, 0:1], RELU)
            warm = ps_h.tile([128, 512], F32, tag="hps")
            for _ in range(32):
                nc.tensor.matmul(warm[:], warm_src[:, 0:128], warm_src[:])

            tok0 = [sum(sched[:i]) * TT for i in range(NBLK)]
            h1s, h2s = {}, {}

            def do_l1(bi):
                W_ = sched[bi] * TT
                xc_t = block_in[bi][0]
                h1 = h_p.tile([128, KC, W_], F16, tag="h1")
                for m in range(KC):
                    ph = ps_h.tile([128, W_], F32, tag="hps")
                    for k in range(KC):
                        nc.tensor.matmul(
                            ph[:],
                            w1_sb[:, k, ts(m, 128)],
                            xc_t[:, k, :],
                            start=(k == 0),
                            stop=(k == KC - 1),
                        )
                    nc.scalar.activation(
                        h1[:, m, :], ph[:], RELU, bias=b1_sb[:, m : m + 1], scale=1.0
                    )
                h1s[bi] = h1

            def do_l2(bi):
                W_ = sched[bi] * TT
                h1 = h1s.pop(bi)
                h2 = h_p.tile([128, KC, W_], F16, tag="h2")
                for m in range(KC):
                    ph = ps_h.tile([128, W_], F32, tag="hps")
                    for k in range(KC):
                        nc.tensor.matmul(
                            ph[:],
                            w2_sb[:, k, ts(m, 128)],
                            h1[:, k, :],
                            start=(k == 0),
                            stop=(k == KC - 1),
                        )
                    nc.scalar.activation(
                        h2[:, m, :], ph[:], RELU, bias=b2_sb[:, m : m + 1], scale=1.0
                    )
                h2s[bi] = h2

            def do_brick(bi):
                # brick head, feature-major: bl[80, W] = Wsc.T @ xbT. Runs
                # between L2 and the comb head so the PE stays busy while
                # h2[3]'s activation drains.
                W_ = sched[bi] * TT
                k0 = tok0[bi]
                xb_t = block_in[bi][1]
                pb = ps_b.tile([NBRICK, W_], F32, tag="bps")
                for k in range(KC):
                    nc.tensor.matmul(
                        pb[:],
                        wsc_sb[:, k, :],
                        xb_t[:, k, :],
                        start=(k == 0),
                        stop=(k == KC - 1),
                    )
                obt = o_p.tile([NBRICK, W_], F16, tag="ob")
                nc.scalar.activation(
                    obt[:], pb[:], IDENT, bias=bbr_sb[:, 0:1], scale=1.0
                )
                nc.sync.dma_start(ob_d[:, k0 : k0 + W_], obt[:])

            def do_comb(bi):
                # comb head, token-major: logits[tok, 920] = h2T.T @ Wh
                nt = sched[bi]
                t0 = tok0[bi] // BL
                h2 = h2s.pop(bi)
                for t in range(nt):
                    pco = ps_c.tile([128, N_COMBINED], F32, tag="cps")
                    for k in range(KC):
                        lhs = h2[:, k, ts(t, 128)]
                        nc.tensor.matmul(
                            pco[:, 0:512],
                            lhs,
                            wh_sb[:, k, 0:512],
                            start=(k == 0),
                            stop=(k == KC - 1),
                        )
                        nc.tensor.matmul(
                            pco[:, 512:N_COMBINED],
                            lhs,
                            wh_sb[:, k, 512:N_COMBINED],
                            start=(k == 0),
                            stop=(k == KC - 1),
                        )
                    oct_ = oc_p.tile([128, N_COMBINED], F16, tag="oc")
                    nc.vector.tensor_add(oct_[:], pco[:], bco_sb[:])
                    nc.sync.dma_start(
                        oc_d[t0 + t * TPB : t0 + (t + 1) * TPB, :, :], oct_[:]
                    )

            # Skewed ramp: L1 of blocks 0-1 first (they only need w1 + xc,
            # the cheapest DMAs), giving w2/wsc/wh time to land before their
            # first consumers. Steady state from block 2 on.
            do_l1(0)
            do_l1(1)
            do_l2(0)
            do_brick(0)
            issue_inputs(2)
            do_l2(1)
            do_comb(0)
            do_comb(1)
            do_brick(1)
            for bi in range(2, NBLK):
                if bi + 1 < NBLK:
                    issue_inputs(bi + 1)
                do_l1(bi)
                do_l2(bi)
                do_brick(bi)
                do_comb(bi)

    nc.compile()
    _BUILD_CACHE["nc"] = nc
    return nc


def _prepare_inputs(inputs):
    """Host-side prep: normalize routing, transpose x to feature-major fp16
    per name, shard over batch, replicate weights."""
    x = np.asarray(inputs["x"], dtype=np.float32)
    readout_x = np.asarray(inputs["readout_x"], dtype=np.int32)
    W1 = np.asarray(inputs["W1"], dtype=np.float32)
    W2 = np.asarray(inputs["W2"], dtype=np.float32)
    Wh = np.asarray(inputs["Wh"], dtype=np.float32)
    Ws = np.asarray(inputs["Ws"], dtype=np.float32)
    Wc = np.asarray(inputs["Wc"], dtype=np.float32)
    b1 = np.asarray(inputs["b1"], dtype=np.float32)
    b2 = np.asarray(inputs["b2"], dtype=np.float32)
    bh = np.asarray(inputs["bh"], dtype=np.float32)
    bs = np.asarray(inputs["bs"], dtype=np.float32)
    bc = np.asarray(inputs["bc"], dtype=np.float32)

    # The kernel hardcodes the cyclic PAD/brick/comb routing. If the actual
    # readout pattern differs, permute x on the host so the device sees the
    # canonical layout (mirrors jnp.nonzero(..., size=ntok) semantics).
    ntok = TS_ * B
    rf = readout_x.reshape(-1)
    canonical = np.array_equal(
        readout_x, np.broadcast_to((np.arange(S, dtype=np.int32) % 3)[:, None], (S, B))
    )
    if not canonical:
        xf = x.reshape(S * B, C)
        xc = np.zeros_like(x).reshape(S * B, C)
        for name_idx in (1, 2):
            idx = np.nonzero(rf == name_idx)[0]
            if idx.shape[0] < ntok:
                idx = np.pad(idx, (0, ntok - idx.shape[0]))
            else:
                idx = idx[:ntok]
            tgt = (3 * (np.arange(ntok) // B) + name_idx) * B + (np.arange(ntok) % B)
            xc[tgt] = xf[idx]
        x = xc.reshape(S, B, C)

    # feature-major fp16 shards: [core, c-part, c-chunk, token]
    def shard_T(y):
        # y: [TS, B, C] fp32 -> [NCORES, 128, KC, NTOK] fp16
        z = y.reshape(TS_, NCORES, BL, KC, 128).transpose(1, 4, 3, 0, 2)
        return np.ascontiguousarray(z.astype(np.float16)).reshape(
            NCORES, 128, KC, NTOK
        )

    xbs = shard_T(x[1::3])
    xcs = shard_T(x[2::3])

    def pack_w(w):
        # [C, width] -> [128 c-part, KC c-chunk, width] fp16
        wid = w.shape[1]
        return np.ascontiguousarray(
            w.astype(np.float16).reshape(KC, 128, wid).transpose(1, 0, 2)
        )

    def pack_w_mmajor(w):
        # [C, C] -> [128 c-part, KC m-chunk, KC k-chunk * 128] fp16:
        # element [p, m, 128k + j] = w[128k + p, 128m + j], so each m-chunk
        # piece is contiguous for its own DMA.
        return np.ascontiguousarray(
            w.astype(np.float16)
            .reshape(KC, 128, KC, 128)
            .transpose(1, 2, 0, 3)
            .reshape(128, KC, C)
        )

    W1h = pack_w(W1)
    W2h = pack_w(W2)
    Whh = pack_w(Wh)
    Wsch = pack_w(np.concatenate([Ws, Wc], axis=1))
    b1t = np.ascontiguousarray(b1.reshape(KC, 128).T)
    b2t = np.ascontiguousarray(b2.reshape(KC, 128).T)
    bco = np.ascontiguousarray(
        np.broadcast_to(bh.astype(np.float16), (128, N_COMBINED))
    )
    bbr = np.ascontiguousarray(np.concatenate([bs, bc]).reshape(NBRICK, 1))

    in_maps = []
    for c in range(NCORES):
        in_maps.append(
            {
                "xc": xcs[c],
                "xb": xbs[c],
                "w1": W1h,
                "w2": W2h,
                "wh": Whh,
                "wsc": Wsch,
                "b1t": b1t,
                "b2t": b2t,
                "bco": bco,
                "bbr": bbr,
            }
        )
    return in_maps


def _run(inputs, trace=False, trace_kwargs=None):
    nc = _build()
    in_maps = _prepare_inputs(inputs)
    res = run_bass_kernel_spmd(
        nc,
        in_maps,
        list(range(NCORES)),
        trace=trace,
        **(trace_kwargs or {}),
    )
    out = np.empty((TS_, B, A), dtype=np.float32)
    for c in range(NCORES):
        sl = slice(c * BL, (c + 1) * BL)
        out[:, sl, NBRICK:] = res.results[c]["oc"]
        out[:, sl, :NBRICK] = (
            res.results[c]["ob"].reshape(NBRICK, TS_, BL).transpose(1, 2, 0)
        )
    return out, res


def kernel(**inputs) -> np.ndarray:
    out, _ = _run(inputs, trace=False)
    return out


if __name__ == "__main__":
    nc = _build()
    print("built OK")


# revision 44
# speedup vs baseline: 1.1347x; 1.1347x over previous
"""Trainium2 Bass kernel for nn_AutoDecoder (moe_routing).

Reference computation (per full input):
  x: [S=3072, B=32, C=512]; rows s%3==1 are "brick" tokens, s%3==2 are
  "combined" tokens (s%3==0 PAD rows are dead). For each (timestep, batch)
  pair:
    brick:  logits[0:80]    = x_brick @ [Ws|Wc]            (+ biases)
    comb:   h = relu(relu(x_comb @ W1 + b1) @ W2 + b2)
            logits[80:1000] = h @ Wh + bh
  out: [TS=1024, B=32, A=1000]

Strategy: data-parallel over batch (4 batch entries per core, 8 cores),
weights replicated. All heavy lifting that is NOT matmul is moved to the
host: x is pre-transposed to feature-major fp16 per readout name, so the
device never transposes, and the MLP + heads are pure back-to-back fp16
matmuls with fp32 PSUM accumulation.

Per core, tokens are processed in blocks of up to 512 (ts,b) pairs:
  L1/L2: feature-major h = relu(W.T @ xT) via ACT (bias fused).
  comb head: token-major logits, stationary = h2 column slices, moving =
    Wh columns (920 streamed cols, no padding waste); DVE adds the
    free-dim bias and narrows to fp16.
  brick head: feature-major [80, W] with Wsc stationary -- 4 matmuls per
    block instead of 16 LDW-bound per-tile matmuls; it also fills the PE
    pipe while the last h2 activation drains. ACT adds the per-partition
    bias. The host transposes the [80, ntok] result back.
Outputs are written fp16 (halves write traffic); the host upcasts.
"""
import sys

if "/opt/trn_rl_repo" not in sys.path:
    sys.path.append("/opt/trn_rl_repo")

import numpy as np

import concourse.bass as bass
from concourse import bacc
import concourse.mybir as mybir
import concourse.tile as tile
from concourse.bass import ts
from concourse.bass_utils import run_bass_kernel_spmd

F32 = mybir.dt.float32
F16 = mybir.dt.float16
RELU = mybir.ActivationFunctionType.Relu
IDENT = mybir.ActivationFunctionType.Identity

# problem dims (hardcoded; kernel.py must be self-contained)
S, B, C = 3072, 32, 512
TS_ = S // 3                      # 1024 timesteps
NUM_SHAPES, NUM_COLORS, N_COMBINED = 64, 16, 920
NBRICK = NUM_SHAPES + NUM_COLORS  # 80
A = NBRICK + N_COMBINED           # 1000
NCORES = 8
BL = B // NCORES                  # 4 batch entries per core
NTOK = TS_ * BL                   # 4096 tokens per name per core
TT = 128                          # tokens per tok-tile
TPB = TT // BL                    # 32 timesteps per tok-tile
KC = C // 128                     # 4 contraction chunks

_BUILD_CACHE = {}


def _build():
    if "nc" in _BUILD_CACHE:
        return _BUILD_CACHE["nc"]
    nc = bacc.Bacc("TRN2", target_bir_lowering=False, debug=False)

    xc_d = nc.declare_dram_parameter("xc", [128, KC, NTOK], F16, isOutput=False)
    xb_d = nc.declare_dram_parameter("xb", [128, KC, NTOK], F16, isOutput=False)
    w1_d = nc.declare_dram_parameter("w1", [128, KC, C], F16, isOutput=False)
    w2_d = nc.declare_dram_parameter("w2", [128, KC, C], F16, isOutput=False)
    wh_d = nc.declare_dram_parameter("wh", [128, KC, N_COMBINED], F16, isOutput=False)
    wsc_d = nc.declare_dram_parameter("wsc", [128, KC, NBRICK], F16, isOutput=False)
    b1_d = nc.declare_dram_parameter("b1t", [128, KC], F32, isOutput=False)
    b2_d = nc.declare_dram_parameter("b2t", [128, KC], F32, isOutput=False)
    bco_d = nc.declare_dram_parameter("bco", [128, N_COMBINED], F16, isOutput=False)
    bbr_d = nc.declare_dram_parameter("bbr", [NBRICK, 1], F32, isOutput=False)
    oc_d = nc.declare_dram_parameter("oc", [TS_, BL, N_COMBINED], F16, isOutput=True)
    ob_d = nc.declare_dram_parameter("ob", [NBRICK, NTOK], F16, isOutput=True)

    with tile.TileContext(nc) as tc:
        with (
            tc.tile_pool(name="const", bufs=1) as const,
            tc.tile_pool(name="xin", bufs=2) as xin_p,
            tc.tile_pool(name="h", bufs=2) as h_p,
            tc.tile_pool(name="osb", bufs=3) as o_p,
            tc.tile_pool(name="occ", bufs=4) as oc_p,
            tc.tile_pool(name="psh", bufs=2, space=bass.MemorySpace.PSUM) as ps_h,
            tc.tile_pool(name="psb", bufs=1, space=bass.MemorySpace.PSUM) as ps_b,
            tc.tile_pool(name="psc", bufs=2, space=bass.MemorySpace.PSUM) as ps_c,
        ):
            # ---- main loop: blocks of nt tok-tiles (128 tokens each) ----
            # Ramp with W=384 blocks: less DMA demand up front, and N>=384
            # matmuls still hide their LDWEIGHTS (unlike N=256).
            sched = [3, 3, 3, 3, 4, 4, 4, 4, 4]
            assert sum(sched) == TS_ // TPB
            NBLK = len(sched)
            block_in = {}

            def issue_inputs(bi, names=(0, 1)):
                # one DMA per name: [128 c-part, KC chunks, W tokens] fp16.
                # xc on the sync queue, xb on gpsimd: parallel issue.
                nt = sched[bi]
                W_ = nt * TT
                k0 = sum(sched[:bi]) * TT
                pair = block_in.setdefault(bi, [None, None])
                if 0 in names:
                    xc_t = xin_p.tile([128, KC, W_], F16, tag="xc")
                    nc.sync.dma_start(xc_t[:], xc_d[:, :, k0 : k0 + W_])
                    pair[0] = xc_t
                if 1 in names:
                    xb_t = xin_p.tile([128, KC, W_], F16, tag="xb")
                    nc.gpsimd.dma_start(xb_t[:], xb_d[:, :, k0 : k0 + W_])
                    pair[1] = xb_t
            # DMA issue order tracks first-use time under the skewed ramp:
            # w1, xc0, xc1, w2, xb0, wsc, wh, xb1, biases. Keeps every
            # transfer ahead of its consumer while the rings are saturated.
            w1_sb = const.tile([128, KC, C], F16, tag="w1")
            nc.scalar.dma_start(w1_sb[:], w1_d[:])
            issue_inputs(0, names=(0,))
            issue_inputs(1, names=(0,))
            w2_sb = const.tile([128, KC, C], F16, tag="w2")
            nc.scalar.dma_start(w2_sb[:], w2_d[:])
            issue_inputs(0, names=(1,))
            wsc_sb = const.tile([128, KC, NBRICK], F16, tag="wsc")
            nc.scalar.dma_start(wsc_sb[:], wsc_d[:])
            wh_sb = const.tile([128, KC, N_COMBINED], F16, tag="wh")
            nc.scalar.dma_start(wh_sb[:], wh_d[:])
            issue_inputs(1, names=(1,))
            b12_sb = const.tile([128, 2 * KC], F32, tag="b12")
            nc.scalar.dma_start(b12_sb[:, 0:KC], b1_d[:, :])
            nc.scalar.dma_start(b12_sb[:, KC : 2 * KC], b2_d[:, :])
            b1_sb = b12_sb[:, 0:KC]
            b2_sb = b12_sb[:, KC : 2 * KC]
            bco_sb = const.tile([128, N_COMBINED], F16, tag="bco")
            nc.scalar.dma_start(bco_sb[:], bco_d[:, :])
            bbr_sb = const.tile([NBRICK, 1], F32, tag="bbr")
            nc.scalar.dma_start(bbr_sb[:], bbr_d[:, :])

            # ---- HAM warmup: keep the PE continuously busy until w1 + xc0
            # have landed (~+15us) so the clock gate is released (K=8/8) and
            # the first real matmuls run warm with no leading gap (a ramp
            # gap re-throttles the PE to half clock). ----
            warm_src = const.tile([128, 512], F16, tag="warm")
            nc.vector.memset(warm_src[:], 0.0)
            # pre-fire the one-time ACT activation-table load
            warm_act = const.tile([1, 1], F32, tag="warmact")
            nc.scalar.activation(warm_act[:], warm_src[0:1, 0:1], RELU)
            warm = ps_h.tile([128, 512], F32, tag="hps")
            for _ in range(30):
                nc.tensor.matmul(warm[:], warm_src[:, 0:128], warm_src[:])

            tok0 = [sum(sched[:i]) * TT for i in range(NBLK)]
            h1s, h2s = {}, {}

            def do_l1(bi):
                W_ = sched[bi] * TT
                xc_t = block_in[bi][0]
                h1 = h_p.tile([128, KC, W_], F16, tag="h1")
                for m in range(KC):
                    ph = ps_h.tile([128, W_], F32, tag="hps")
                    for k in range(KC):
                        nc.tensor.matmul(
                            ph[:],
                            w1_sb[:, k, ts(m, 128)],
                            xc_t[:, k, :],
                            start=(k == 0),
                            stop=(k == KC - 1),
                        )
                    nc.scalar.activation(
                        h1[:, m, :], ph[:], RELU, bias=b1_sb[:, m : m + 1], scale=1.0
                    )
                h1s[bi] = h1

            def do_l2(bi):
                W_ = sched[bi] * TT
                h1 = h1s.pop(bi)
                h2 = h_p.tile([128, KC, W_], F16, tag="h2")
                for m in range(KC):
                    ph = ps_h.tile([128, W_], F32, tag="hps")
                    for k in range(KC):
                        nc.tensor.matmul(
                            ph[:],
                            w2_sb[:, k, ts(m, 128)],
                            h1[:, k, :],
                            start=(k == 0),
                            stop=(k == KC - 1),
                        )
                    nc.scalar.activation(
                        h2[:, m, :], ph[:], RELU, bias=b2_sb[:, m : m + 1], scale=1.0
                    )
                h2s[bi] = h2

            def do_brick(bi):
                # brick head, feature-major: bl[80, W] = Wsc.T @ xbT. Runs
                # between L2 and the comb head so the PE stays busy while
                # h2[3]'s activation drains.
                W_ = sched[bi] * TT
                k0 = tok0[bi]
                xb_t = block_in[bi][1]
                pb = ps_b.tile([NBRICK, W_], F32, tag="bps")
                for k in range(KC):
                    nc.tensor.matmul(
                        pb[:],
                        wsc_sb[:, k, :],
                        xb_t[:, k, :],
                        start=(k == 0),
                        stop=(k == KC - 1),
                    )
                obt = o_p.tile([NBRICK, W_], F16, tag="ob")
                nc.scalar.activation(
                    obt[:], pb[:], IDENT, bias=bbr_sb[:, 0:1], scale=1.0
                )
                nc.sync.dma_start(ob_d[:, k0 : k0 + W_], obt[:])

            def do_comb(bi):
                # comb head, token-major: logits[tok, 920] = h2T.T @ Wh
                nt = sched[bi]
                t0 = tok0[bi] // BL
                h2 = h2s.pop(bi)
                for t in range(nt):
                    pco = ps_c.tile([128, N_COMBINED], F32, tag="cps")
                    last = bi == NBLK - 1 and t == nt - 1
                    if not last:
                        for k in range(KC):
                            lhs = h2[:, k, ts(t, 128)]
                            nc.tensor.matmul(
                                pco[:, 0:512],
                                lhs,
                                wh_sb[:, k, 0:512],
                                start=(k == 0),
                                stop=(k == KC - 1),
                            )
                            nc.tensor.matmul(
                                pco[:, 512:N_COMBINED],
                                lhs,
                                wh_sb[:, k, 512:N_COMBINED],
                                start=(k == 0),
                                stop=(k == KC - 1),
                            )
                        oct_ = oc_p.tile([128, N_COMBINED], F16, tag="oc")
                        nc.vector.tensor_add(oct_[:], pco[:], bco_sb[:])
                        nc.sync.dma_start(
                            oc_d[t0 + t * TPB : t0 + (t + 1) * TPB, :, :], oct_[:]
                        )
                    else:
                        # final tile: finish the 0:512 column group first so
                        # its bias-add + writeback overlap the 512:920
                        # matmuls, shortening the end-of-kernel drain
                        for k in range(KC):
                            nc.tensor.matmul(
                                pco[:, 0:512],
                                h2[:, k, ts(t, 128)],
                                wh_sb[:, k, 0:512],
                                start=(k == 0),
                                stop=(k == KC - 1),
                            )
                        oct_ = oc_p.tile([128, N_COMBINED], F16, tag="oc")
                        nc.vector.tensor_add(
                            oct_[:, 0:512], pco[:, 0:512], bco_sb[:, 0:512]
                        )
                        nc.sync.dma_start(
                            oc_d[t0 + t * TPB : t0 + (t + 1) * TPB, :, 0:512],
                            oct_[:, 0:512],
                        )
                        for k in range(KC):
                            nc.tensor.matmul(
                                pco[:, 512:N_COMBINED],
                                h2[:, k, ts(t, 128)],
                                wh_sb[:, k, 512:N_COMBINED],
                                start=(k == 0),
                                stop=(k == KC - 1),
                            )
                        nc.vector.tensor_add(
                            oct_[:, 512:N_COMBINED],
                            pco[:, 512:N_COMBINED],
                            bco_sb[:, 512:N_COMBINED],
                        )
                        nc.sync.dma_start(
                            oc_d[t0 + t * TPB : t0 + (t + 1) * TPB, :, 512:N_COMBINED],
                            oct_[:, 512:N_COMBINED],
                        )

            # Skewed ramp: L1 of blocks 0-1 first (they only need w1 + xc,
            # the cheapest DMAs), giving w2/wsc/wh time to land before their
            # first consumers. Steady state from block 2 on.
            do_l1(0)
            do_l1(1)
            do_l2(0)
            do_brick(0)
            issue_inputs(2)
            do_l2(1)
            do_comb(0)
            do_comb(1)
            do_brick(1)
            for bi in range(2, NBLK):
                if bi + 1 < NBLK:
                    issue_inputs(bi + 1)
                do_l1(bi)
                do_l2(bi)
                do_brick(bi)
                do_comb(bi)

    nc.compile()
    _BUILD_CACHE["nc"] = nc
    return nc


def _prepare_inputs(inputs):
    """Host-side prep: normalize routing, transpose x to feature-major fp16
    per name, shard over batch, replicate weights."""
    x = np.asarray(inputs["x"], dtype=np.float32)
    readout_x = np.asarray(inputs["readout_x"], dtype=np.int32)
    W1 = np.asarray(inputs["W1"], dtype=np.float32)
    W2 = np.asarray(inputs["W2"], dtype=np.float32)
    Wh = np.asarray(inputs["Wh"], dtype=np.float32)
    Ws = np.asarray(inputs["Ws"], dtype=np.float32)
    Wc = np.asarray(inputs["Wc"], dtype=np.float32)
    b1 = np.asarray(inputs["b1"], dtype=np.float32)
    b2 = np.asarray(inputs["b2"], dtype=np.float32)
    bh = np.asarray(inputs["bh"], dtype=np.float32)
    bs = np.asarray(inputs["bs"], dtype=np.float32)
    bc = np.asarray(inputs["bc"], dtype=np.float32)

    # The kernel hardcodes the cyclic PAD/brick/comb routing. If the actual
    # readout pattern differs, permute x on the host so the device sees the
    # canonical layout (mirrors jnp.nonzero(..., size=ntok) semantics).
    ntok = TS_ * B
    rf = readout_x.reshape(-1)
    canonical = np.array_equal(
        readout_x, np.broadcast_to((np.arange(S, dtype=np.int32) % 3)[:, None], (S, B))
    )
    if not canonical:
        xf = x.reshape(S * B, C)
        xc = np.zeros_like(x).reshape(S * B, C)
        for name_idx in (1, 2):
            idx = np.nonzero(rf == name_idx)[0]
            if idx.shape[0] < ntok:
                idx = np.pad(idx, (0, ntok - idx.shape[0]))
            else:
                idx = idx[:ntok]
            tgt = (3 * (np.arange(ntok) // B) + name_idx) * B + (np.arange(ntok) % B)
            xc[tgt] = xf[idx]
        x = xc.reshape(S, B, C)

    # feature-major fp16 shards: [core, c-part, c-chunk, token]
    def shard_T(y):
        # y: [TS, B, C] fp32 -> [NCORES, 128, KC, NTOK] fp16
        z = y.reshape(TS_, NCORES, BL, KC, 128).transpose(1, 4, 3, 0, 2)
        return np.ascontiguousarray(z.astype(np.float16)).reshape(
            NCORES, 128, KC, NTOK
        )

    xbs = shard_T(x[1::3])
    xcs = shard_T(x[2::3])

    def pack_w(w):
        # [C, width] -> [128 c-part, KC c-chunk, width] fp16
        wid = w.shape[1]
        return np.ascontiguousarray(
            w.astype(np.float16).reshape(KC, 128, wid).transpose(1, 0, 2)
        )

    def pack_w_mmajor(w):
        # [C, C] -> [128 c-part, KC m-chunk, KC k-chunk * 128] fp16:
        # element [p, m, 128k + j] = w[128k + p, 128m + j], so each m-chunk
        # piece is contiguous for its own DMA.
        return np.ascontiguousarray(
            w.astype(np.float16)
            .reshape(KC, 128, KC, 128)
            .transpose(1, 2, 0, 3)
            .reshape(128, KC, C)
        )

    W1h = pack_w(W1)
    W2h = pack_w(W2)
    Whh = pack_w(Wh)
    Wsch = pack_w(np.concatenate([Ws, Wc], axis=1))
    b1t = np.ascontiguousarray(b1.reshape(KC, 128).T)
    b2t = np.ascontiguousarray(b2.reshape(KC, 128).T)
    bco = np.ascontiguousarray(
        np.broadcast_to(bh.astype(np.float16), (128, N_COMBINED))
    )
    bbr = np.ascontiguousarray(np.concatenate([bs, bc]).reshape(NBRICK, 1))

    in_maps = []
    for c in range(NCORES):
        in_maps.append(
            {
                "xc": xcs[c],
                "xb": xbs[c],
                "w1": W1h,
                "w2": W2h,
                "wh": Whh,
                "wsc": Wsch,
                "b1t": b1t,
                "b2t": b2t,
                "bco": bco,
                "bbr": bbr,
            }
        )
    return in_maps


def _run(inputs, trace=False, trace_kwargs=None):
    nc = _build()
    in_maps = _prepare_inputs(inputs)
    res = run_bass_kernel_spmd(
        nc,
        in_maps,
        list(range(NCORES)),
        trace=trace,
        **(trace_kwargs or {}),
    )
    out = np.empty((TS_, B, A), dtype=np.float32)
    for c in range(NCORES):
        sl = slice(c * BL, (c + 1) * BL)
        out[:, sl, NBRICK:] = res.results[c]["oc"]
        out[:, sl, :NBRICK] = (
            res.results[c]["ob"].reshape(NBRICK, TS_, BL).transpose(1, 2, 0)
        )
    return out, res


def kernel(**inputs) -> np.ndarray:
    out, _ = _run(inputs, trace=False)
    return out


if __name__ == "__main__":
    nc = _build()
    print("built OK")
